# revision 22
# baseline (speedup 1.0000x reference)
"""
Trainium2 Bass kernel: MultiStepLIF (T=4) -> depthwise 3x3 conv -> BatchNorm2d
(training-mode batch stats), data-parallel over batch across 8 NeuronCores.

Contract: kernel(**inputs) takes FULL numpy inputs
    x: [4, 16, 384, 32, 32] f32, w: [384, 1, 3, 3] f32, gamma/beta: [384] f32
and returns the FULL output [4, 16, 384, 32, 32] f32.

Per core (batch shard of 2):
  - LIF scan in doubled-membrane form u_t = u_{t-1}*0.5 + x_t (one DVE
    scalar_tensor_tensor per step; *0.5 is exact so u/2 reproduces the
    reference fp32 membrane bit-for-bit); hard reset u <- (u<2)*u in one STT.
  - The spike threshold (u >= 2, DVE is_ge) writes DIRECTLY into a
    zero-bordered padded grid [1 + 34*33] f16 via a strided 2-D AP: 33-wide
    rows (col 32 always zero) + zero rows top/bottom. Any 3x3-shifted window
    is then a CONTIGUOUS 1-D slice with exact zero padding at row wraps.
  - Depthwise conv: f16 diagonal weights, plain matmuls (fp8 DoubleRow
    pair-split measured ~2x SLOWER on this hardware and less accurate).
    Per 32-row tile: one 4-bank PSUM tile, 4x 8-row segments, k-outer loop
    (one stationary load per tap; 4 segment matmuls per load, PSUM
    accumulation interleaved across banks).
  - PSUM evacuation: one ScalarE Copy per tile with fused accum_out -> BN
    sums; sum of squares via ScalarE Square activation with accum_out.
  - Sync-BN: AllReduce-add of [128, 6] per-channel partial sums (required:
    per-shard BN stats measure 7.1e-2 rel err vs the 2e-2 gate).
  - BN parameters on-device (rsqrt + one Newton step); BN apply y*a + b
    split across DVE and Pool (gpsimd) engines in place on the f16 staging
    buffer, then one strided DMA per channel chunk stores the f16 output
    (host upcasts to f32; ~6e-4 rel rounding against the 2e-2 gate).

Engine balance (measured, marginal per-execution on HW ~40-48 us):
  DVE: LIF update/reset, spike thresholds, half of sum-of-squares, half of
    BN apply.  ScalarE: PSUM evacuation + other half of sum-of-squares
    (Square) with fused accums.  Pool: other half of BN apply.
  PE: 864 f16 matmuls (~6+ cols/cycle).
  DMA: 12.6 MB in + 6.3 MB out at ~1.36 TB/s/core (measured) ~ 14 us.

`repeat` unrolls the ENTIRE body (input DMA ... collective ... output DMA)
N times with double-buffered staging so (t_R - t_1)/(R-1) on interleaved
dispatches measures one full steady-state on-device execution with the
axon-tunnel RTT (~75-90 ms per blocking dispatch regardless of content)
cancelled out.
"""

import numpy as np

# ---- problem constants (hardcoded; kernel must be self-contained) ----
T = 4
B = 16
C = 384
H = 32
W = 32
HW = H * W
NCORES = 8
BS = B // NCORES          # batch per core = 2
NCHUNK = C // 128         # 3 channel chunks of 128
NTILE = NCHUNK * BS * T   # 24 [128, 1024] output tiles per core
NTOT = T * B * HW         # 65536 samples per channel for BN stats
BN_EPS = 1e-5
NSEG = 4                  # conv row-segments per 32-row tile (8 rows each)
SROWS = H // NSEG         # 8
ROWB = W + 1              # 33: padded row pitch (one zero col)
SEGN = SROWS * ROWB - 1   # 263 matmul output columns per segment
SPLEN = 1 + 34 * ROWB     # leading zero elem + 34 padded rows

_CACHE = {}


def build_program(n_cores=NCORES, with_collective=True, taps=9, repeat=1,
                  ssq_eng="vs", reset_eng="v", evac_eng="s", bn_cycle="vp",
                  spike_eng="v", nsp=6, morder="k", evac_seg=False,
                  batch_b=True):
    import concourse.bass as bass
    import concourse.bacc as bacc
    import concourse.tile as tile
    import concourse.mybir as mybir

    f32 = mybir.dt.float32
    f16 = mybir.dt.float16
    AL = mybir.AluOpType
    AF = mybir.ActivationFunctionType
    AX = mybir.AxisListType

    nc = bacc.Bacc("TRN2", target_bir_lowering=False, debug=False,
                   num_devices=n_cores)

    # channel-major DRAM layouts: the host pre-transposes x (and
    # post-transposes y) so every DMA is a fully contiguous 2-dim transfer
    # instead of a 4-dim strided one with 4KB runs
    x_d = nc.dram_tensor("x", [128, NCHUNK, T, BS, HW], f32,
                         kind="ExternalInput").ap()
    wd_d = nc.dram_tensor("wd", [128, NCHUNK * 9, 128], f16,
                          kind="ExternalInput").ap()
    gb_d = nc.dram_tensor("gb", [128, 6], f32, kind="ExternalInput").ap()
    y_d = nc.dram_tensor("y", [128, NCHUNK, T, BS, HW], f16,
                         kind="ExternalOutput").ap()

    def eng(c):
        return {"v": nc.vector, "s": nc.scalar, "p": nc.gpsimd}[c]

    def tidx(ch, b, t):
        # t-major within a chunk: matches the [T, BS] order of the
        # channel-major DRAM layout so chunk DMAs stay contiguous
        return (ch * T + t) * BS + b

    with tile.TileContext(nc) as tc:
        with (
            tc.tile_pool(name="const", bufs=1) as cpool,
            tc.tile_pool(name="w", bufs=2) as wpool,
            tc.tile_pool(name="stage", bufs=2) as spool,
            tc.tile_pool(name="xin", bufs=2) as xpool,
            tc.tile_pool(name="lif", bufs=(2 if batch_b else 3)) as lpool,
            tc.tile_pool(name="outp",
                         bufs=(3 if evac_seg else 2 if batch_b else 4)
                         ) as opool,
            tc.tile_pool(name="ps", bufs=2, space="PSUM") as pspool,
            tc.tile_pool(name="dram", bufs=1, space="DRAM") as dpool,
        ):
            # persistent padded spike grids (borders zeroed once; interior is
            # fully overwritten each use, borders are zero-invariant).
            # batch_b: each grid holds BOTH b-planes side by side so one DVE
            # op per LIF step covers [128, BS*HW].
            if batch_b:
                nsp = nsp // 2
            sp_bufs = []
            for i in range(nsp):
                spb = cpool.tile([128, (BS * SPLEN) if batch_b else SPLEN],
                                 f16, name=f"spb{i}")
                nc.vector.memset(spb[:], 0.0)
                sp_bufs.append(spb)

            it = 0
            for rep in range(repeat):
                # ---- weights + gamma/beta ----
                wsb = wpool.tile([128, NCHUNK * 9, 128], f16, tag="wsb",
                                 name="wsb")
                nc.sync.dma_start(out=wsb[:], in_=wd_d[:])
                gbsb = wpool.tile([128, 6], f32, tag="gbsb", name="gbsb")
                nc.sync.dma_start(out=gbsb[:], in_=gb_d[:])
                # conv output staging + stats, double-buffered across reps so
                # rep r+1's phase 1 overlaps rep r's BN apply + output DMA
                y_all = spool.tile([128, NTILE, HW], f16, tag="y_all",
                                   name="y_all")
                nsum = NTILE * NSEG if evac_seg else NTILE
                ssum = spool.tile([128, nsum], f32, tag="ssum", name="ssum")
                ssq = spool.tile([128, NTILE], f32, tag="ssq", name="ssq")
                loc = spool.tile([128, 6], f32, tag="loc", name="loc")
                gsum = spool.tile([128, 6], f32, tag="gsum", name="gsum")
                prm = spool.tile([128, 48], f32, tag="prm", name="prm")

                # ---- phase 1: LIF + depthwise conv + BN partial sums ----
                for ch in range(NCHUNK):
                    xsb = xpool.tile([128, T, BS, HW], f32, tag="xsb",
                                     name="xsb")
                    nc.sync.dma_start(out=xsb[:], in_=x_d[:, ch])
                    if batch_b:
                        # both b-chains advance in lockstep: one DVE op per
                        # LIF step over [128, BS*HW]; spikes land in a
                        # doubled padded grid (b-plane stride SPLEN)
                        u_prev = None
                        for t in range(T):
                            xt2 = xsb[:, t]          # [128, BS, HW]
                            if t == 0:
                                u2 = xt2
                            else:
                                u2t = lpool.tile([128, BS, HW], f32, tag="u",
                                                 name="u")
                                nc.vector.scalar_tensor_tensor(
                                    out=u2t[:], in0=u_prev[:], scalar=0.5,
                                    in1=xt2[:], op0=AL.mult, op1=AL.add)
                                u2 = u2t
                            sp = sp_bufs[it % nsp]
                            it += 1
                            grid2 = bass.AP(
                                tensor=sp.tensor, offset=sp.offset + 1 + ROWB,
                                ap=[sp.ap[0], [SPLEN, BS], [ROWB, H],
                                    [1, W]])
                            eng(spike_eng).tensor_scalar(grid2, u2[:], 2.0,
                                                         None, AL.is_ge)
                            if t < T - 1:
                                un = lpool.tile([128, BS, HW], f32, tag="u",
                                                name="u")
                                eng(reset_eng).scalar_tensor_tensor(
                                    out=un[:], in0=u2[:], scalar=2.0,
                                    in1=u2[:], op0=AL.is_lt, op1=AL.mult)
                                u_prev = un
                            for b in range(BS):
                                ti = tidx(ch, b, t)
                                ps = pspool.tile([128, NSEG, 512], f32,
                                                 tag="ps", name="ps")
                                for k in range(taps):
                                    dh, dw = k // 3, k % 3
                                    lhsT = wsb[:, ch * 9 + k, :]
                                    for si in range(NSEG):
                                        off = (b * SPLEN + 1
                                               + (si * SROWS + dh) * ROWB
                                               + dw - 1)
                                        rhs = sp[:, off:off + SEGN]
                                        nc.tensor.matmul(
                                            out=ps[:, si, 0:SEGN], lhsT=lhsT,
                                            rhs=rhs, start=(k == 0),
                                            stop=(k == taps - 1))
                                ps4 = bass.AP(
                                    tensor=ps.tensor, offset=ps.offset,
                                    ap=[ps.ap[0], [512, NSEG], [ROWB, SROWS],
                                        [1, W]])
                                y4 = y_all[:, ti, :].rearrange(
                                    "p (s r w) -> p s r w", s=NSEG, r=SROWS)
                                nc.scalar.activation(
                                    out=y4, in_=ps4, func=AF.Copy,
                                    accum_out=ssum[:, ti:ti + 1])
                                se = ssq_eng[ti % len(ssq_eng)]
                                sc = opool.tile([128, HW], f16, tag="sc",
                                                name="sc")
                                if se == "s":
                                    nc.scalar.activation(
                                        out=sc[:], in_=y_all[:, ti, :],
                                        func=AF.Square,
                                        accum_out=ssq[:, ti:ti + 1])
                                else:
                                    eng(se).scalar_tensor_tensor(
                                        out=sc[:], in0=y_all[:, ti, :],
                                        scalar=1.0, in1=y_all[:, ti, :],
                                        op0=AL.mult, op1=AL.mult,
                                        accum_out=ssq[:, ti:ti + 1])
                        continue
                    for b in range(BS):
                        u_prev = None
                        for t in range(T):
                            xt = xsb[:, t, b, :]
                            if t == 0:
                                u = xt  # u_0 = x_0 (threshold doubles to 2.0)
                            else:
                                u = lpool.tile([128, HW], f32, tag="u",
                                               name="u")
                                nc.vector.scalar_tensor_tensor(
                                    out=u[:], in0=u_prev[:], scalar=0.5,
                                    in1=xt[:], op0=AL.mult, op1=AL.add)

                            # spike straight into padded grid rows 1..32,
                            # cols 0..31 (row pitch 33; col 32 stays zero)
                            sp = sp_bufs[it % nsp]
                            it += 1
                            grid = bass.AP(
                                tensor=sp.tensor, offset=sp.offset + 1 + ROWB,
                                ap=[sp.ap[0], [ROWB, H], [1, W]])
                            eng(spike_eng).tensor_scalar(grid, u[:], 2.0,
                                                         None, AL.is_ge)

                            if t < T - 1:
                                # hard reset in one op: u <- (u < 2) * u
                                un = lpool.tile([128, HW], f32, tag="u",
                                                name="u")
                                eng(reset_eng).scalar_tensor_tensor(
                                    out=un[:], in0=u[:], scalar=2.0, in1=u[:],
                                    op0=AL.is_lt, op1=AL.mult)
                                u_prev = un

                            # conv: k-outer (one weight load per tap), 4
                            # segments accumulate in one 4-bank PSUM tile
                            ti = tidx(ch, b, t)
                            ps = pspool.tile([128, NSEG, 512], f32, tag="ps",
                                             name="ps")
                            if evac_seg:
                                # si-outer: each segment's 9 taps complete,
                                # then its evacuation overlaps the next
                                # segment's matmuls (earlier PSUM handoff)
                                for si in range(NSEG):
                                    for k in range(taps):
                                        dh, dw = k // 3, k % 3
                                        lhsT = wsb[:, ch * 9 + k, :]
                                        off = (1 + (si * SROWS + dh) * ROWB
                                               + dw - 1)
                                        rhs = sp[:, off:off + SEGN]
                                        nc.tensor.matmul(
                                            out=ps[:, si, 0:SEGN], lhsT=lhsT,
                                            rhs=rhs, start=(k == 0),
                                            stop=(k == taps - 1))
                                    psg = bass.AP(
                                        tensor=ps.tensor,
                                        offset=ps.offset + si * 512,
                                        ap=[ps.ap[0], [ROWB, SROWS], [1, W]])
                                    ysg = y_all[
                                        :, ti,
                                        si * SROWS * W:(si + 1) * SROWS * W
                                    ].rearrange("p (r w) -> p r w", r=SROWS)
                                    nc.scalar.activation(
                                        out=ysg, in_=psg, func=AF.Copy,
                                        accum_out=ssum[
                                            :, ti * NSEG + si:
                                            ti * NSEG + si + 1])
                            else:
                                if morder == "k":
                                    order = [(k, si) for k in range(taps)
                                             for si in range(NSEG)]
                                else:
                                    order = [(k, si) for si in range(NSEG)
                                             for k in range(taps)]
                                for k, si in order:
                                    dh, dw = k // 3, k % 3
                                    lhsT = wsb[:, ch * 9 + k, :]
                                    off = 1 + (si * SROWS + dh) * ROWB + dw - 1
                                    rhs = sp[:, off:off + SEGN]
                                    nc.tensor.matmul(
                                        out=ps[:, si, 0:SEGN], lhsT=lhsT,
                                        rhs=rhs, start=(k == 0),
                                        stop=(k == taps - 1))
                                # evacuate tile (skip junk col via 4-D AP)
                                # with fused per-channel sum
                                ps4 = bass.AP(
                                    tensor=ps.tensor, offset=ps.offset,
                                    ap=[ps.ap[0], [512, NSEG], [ROWB, SROWS],
                                        [1, W]])
                                y4 = y_all[:, ti, :].rearrange(
                                    "p (s r w) -> p s r w", s=NSEG, r=SROWS)
                                ee = evac_eng[ti % len(evac_eng)]
                                if ee == "s":
                                    nc.scalar.activation(
                                        out=y4, in_=ps4, func=AF.Copy,
                                        accum_out=ssum[:, ti:ti + 1])
                                else:
                                    eng(ee).tensor_scalar(
                                        y4, ps4, 1.0, None, AL.mult,
                                        accum_out=ssum[:, ti:ti + 1])
                            # sum of squares (engine cycled per tile)
                            se = ssq_eng[ti % len(ssq_eng)]
                            sc = opool.tile([128, HW], f16, tag="sc",
                                            name="sc")
                            if se == "s":
                                nc.scalar.activation(
                                    out=sc[:], in_=y_all[:, ti, :],
                                    func=AF.Square,
                                    accum_out=ssq[:, ti:ti + 1])
                            else:
                                eng(se).scalar_tensor_tensor(
                                    out=sc[:], in0=y_all[:, ti, :], scalar=1.0,
                                    in1=y_all[:, ti, :], op0=AL.mult,
                                    op1=AL.mult, accum_out=ssq[:, ti:ti + 1])

                # ---- phase 2: reduce partials, sync-BN all-reduce ----
                NQ = BS * T
                NQS = NQ * NSEG if evac_seg else NQ
                for ch in range(NCHUNK):
                    nc.vector.tensor_reduce(
                        out=loc[:, ch:ch + 1],
                        in_=ssum[:, ch * NQS:(ch + 1) * NQS], axis=AX.X,
                        op=AL.add)
                    nc.vector.tensor_reduce(
                        out=loc[:, 3 + ch:4 + ch],
                        in_=ssq[:, ch * NQ:(ch + 1) * NQ], axis=AX.X,
                        op=AL.add)

                if with_collective:
                    cin = dpool.tile([128, 6], f32, tag="cin", name="cin")
                    cout = dpool.tile([128, 6], f32, tag="cout", name="cout")
                    nc.gpsimd.dma_start(out=cin[:], in_=loc[:])
                    nc.gpsimd.collective_compute(
                        "AllReduce", AL.add,
                        replica_groups=[list(range(n_cores))],
                        ins=[cin.opt()], outs=[cout.opt()])
                    nc.gpsimd.dma_start(out=gsum[:], in_=cout[:])
                else:
                    nc.vector.tensor_copy(out=gsum[:], in_=loc[:])

                # ---- phase 3: BN parameters (tiny [128,3] ops) ----
                mu, m2 = prm[:, 0:3], prm[:, 3:6]
                sq, var = prm[:, 6:9], prm[:, 9:12]
                veps, sv = prm[:, 12:15], prm[:, 15:18]
                r0_, r0s = prm[:, 18:21], prm[:, 21:24]
                xr, tcr = prm[:, 24:27], prm[:, 27:30]
                r1_, aa = prm[:, 30:33], prm[:, 33:36]
                t2, bb = prm[:, 36:39], prm[:, 39:42]
                inv_n = 1.0 / float(NTOT)
                nc.vector.tensor_scalar(mu, gsum[:, 0:3], inv_n, None,
                                        AL.mult)
                nc.vector.tensor_scalar(m2, gsum[:, 3:6], inv_n, None,
                                        AL.mult)
                nc.vector.tensor_tensor(out=sq, in0=mu, in1=mu, op=AL.mult)
                nc.vector.tensor_tensor(out=var, in0=m2, in1=sq,
                                        op=AL.subtract)
                nc.vector.tensor_scalar(veps, var, BN_EPS, None, AL.add)
                nc.scalar.activation(out=sv, in_=veps, func=AF.Sqrt)
                nc.vector.reciprocal(out=r0_, in_=sv)
                # one Newton step: r1 = r0*(1.5 - 0.5*x*r0^2)
                nc.vector.tensor_tensor(out=r0s, in0=r0_, in1=r0_, op=AL.mult)
                nc.vector.tensor_tensor(out=xr, in0=veps, in1=r0s, op=AL.mult)
                nc.vector.tensor_scalar(tcr, xr, -0.5, 1.5, AL.mult, AL.add)
                nc.vector.tensor_tensor(out=r1_, in0=r0_, in1=tcr, op=AL.mult)
                # a = r1*gamma ; b = beta - mu*a
                nc.vector.tensor_tensor(out=aa, in0=r1_, in1=gbsb[:, 0:3],
                                        op=AL.mult)
                nc.vector.tensor_tensor(out=t2, in0=mu, in1=aa, op=AL.mult)
                nc.vector.tensor_tensor(out=bb, in0=gbsb[:, 3:6], in1=t2,
                                        op=AL.subtract)

                # ---- phase 4: BN apply in place (engines cycled), then one
                # strided DMA per channel chunk ----
                n = 0
                for ch in range(NCHUNK):
                    for ti in range(ch * BS * T, (ch + 1) * BS * T):
                        e = bn_cycle[n % len(bn_cycle)]
                        if e == "s":
                            nc.scalar.activation(
                                out=y_all[:, ti, :], in_=y_all[:, ti, :],
                                func=AF.Identity,
                                bias=prm[:, 39 + ch:40 + ch],
                                scale=prm[:, 33 + ch:34 + ch])
                        else:
                            eng(e).tensor_scalar(
                                y_all[:, ti, :], y_all[:, ti, :],
                                prm[:, 33 + ch:34 + ch],
                                prm[:, 39 + ch:40 + ch],
                                AL.mult, AL.add)
                        n += 1
                    nc.sync.dma_start(
                        out=y_d[:, ch],
                        in_=y_all[:, ch * BS * T:(ch + 1) * BS * T, :])
    nc.compile()
    return nc


def _host_prep(x, w, gamma, beta):
    """Shard/transform the full inputs into per-core in_maps."""
    x = np.asarray(x, dtype=np.float32).reshape(T, B, C, HW)
    w = np.asarray(w, dtype=np.float32)
    gamma = np.asarray(gamma, dtype=np.float32)
    beta = np.asarray(beta, dtype=np.float32)

    w9 = w.reshape(C, 9).astype(np.float16)
    idx = np.arange(128)
    wd = np.zeros((128, NCHUNK * 9, 128), dtype=np.float16)
    for ch in range(NCHUNK):
        for k in range(9):
            wd[idx, ch * 9 + k, idx] = w9[ch * 128:(ch + 1) * 128, k]

    gb = np.zeros((128, 6), dtype=np.float32)
    gb[:, 0:3] = gamma.reshape(NCHUNK, 128).T
    gb[:, 3:6] = beta.reshape(NCHUNK, 128).T

    # channel-major: [128(part), NCHUNK, T, B, HW] so device DMAs are
    # fully contiguous per chunk
    xt = x.reshape(T, B, NCHUNK, 128, HW).transpose(3, 2, 0, 1, 4)
    in_maps = []
    for i in range(NCORES):
        xi = np.ascontiguousarray(xt[:, :, :, i * BS:(i + 1) * BS])
        in_maps.append({"x": xi, "wd": wd, "gb": gb})
    return in_maps


def kernel(x, w, gamma, beta):
    from concourse.bass_utils import run_bass_kernel_spmd

    if "nc" not in _CACHE:
        _CACHE["nc"] = build_program()
    nc = _CACHE["nc"]

    in_maps = _host_prep(x, w, gamma, beta)
    res = run_bass_kernel_spmd(nc, in_maps, core_ids=list(range(NCORES)))

    out = np.empty((T, B, C, HW), dtype=np.float32)
    for i in range(NCORES):
        yi = res.results[i]["y"]  # [128, NCHUNK, T, BS, HW] f16
        out[:, i * BS:(i + 1) * BS] = (
            yi.transpose(2, 3, 1, 0, 4).reshape(T, BS, C, HW))  # f32 upcast
    return out.reshape(T, B, C, H, W)
